# revision 23
# baseline (speedup 1.0000x reference)
"""Trainium2 8-core kernel for causal GQA attention (nn_Attention_90967407329949).

v1 architecture (no kv dedup) + stall fixes:
 - ag_in writes on gpsimd ring (off the big-load lanes)
 - oproj schedule shifted one segment later (dodges the slow first AllGather)
 - tail reordered: oproj(3,0) interleaved with att(3,1); only oproj(3,1)
   after the last collective
 - startup: wq chunk0 + xblk0 first on their lanes

Distribution: tensor-parallel over query heads (2 q-heads + shared kv-head
per core); per (batch, half) AllGathers of attention outputs; each core
computes a 256-column slice of the output projection.

V tiles are transposed on the PE (identity matmul), NOT dma_start_transpose
(Tile serializes DMA transposes with collectives).
"""

import numpy as np
import ml_dtypes

import concourse.bacc as bacc
import concourse.mybir as mybir
import concourse.tile as tile
from concourse.bass_utils import run_bass_kernel_spmd

BF16 = mybir.dt.bfloat16
F32 = mybir.dt.float32

N_CORES = 8
B = 4
N = 1024
NT = B * N
D = 2048
DH = 128
KT = D // 128
SCALE = 1.0 / np.sqrt(DH)

_NC_CACHE = {}


def build_nc():
    if "nc" in _NC_CACHE:
        return _NC_CACHE["nc"]
    nc = bacc.Bacc("TRN2", target_bir_lowering=False, debug=False, num_devices=N_CORES)

    xt = nc.dram_tensor("xt", [D, NT], BF16, kind="ExternalInput")
    wq = nc.dram_tensor("wq", [128, KT, 256], BF16, kind="ExternalInput")
    wk = nc.dram_tensor("wk", [128, KT, 128], BF16, kind="ExternalInput")
    wv = nc.dram_tensor("wv", [128, KT, 128], BF16, kind="ExternalInput")
    wo = nc.dram_tensor("wo", [128, KT, 256], BF16, kind="ExternalInput")
    cost = nc.dram_tensor("cost", [128, N], BF16, kind="ExternalInput")
    sint = nc.dram_tensor("sint", [128, N], BF16, kind="ExternalInput")
    mask = nc.dram_tensor("mask", [128, 2048], BF16, kind="ExternalInput")
    ident = nc.dram_tensor("ident", [128, 128], BF16, kind="ExternalInput")
    out = nc.dram_tensor("out", [256, NT], BF16, kind="ExternalOutput")

    SEGS = [(b, ib) for b in range(B) for ib in range(2)]
    ag_in = {s: nc.dram_tensor(f"agi{s[0]}{s[1]}", [256, 512], BF16) for s in SEGS}
    ag_out = {s: nc.dram_tensor(f"ago{s[0]}{s[1]}", [D, 512], BF16,
                                addr_space="Shared") for s in SEGS}
    ag_out_r = {s: t.rearrange("(t p) n -> p t n", p=128) for s, t in ag_out.items()}

    with tile.TileContext(nc) as tc:
        with (
            tc.tile_pool(name="const", bufs=1) as constp,
            tc.tile_pool(name="persist", bufs=1) as persist,
            tc.tile_pool(name="xtp", bufs=3) as xtp,
            tc.tile_pool(name="qkraw", bufs=2) as qkrawp,
            tc.tile_pool(name="rope", bufs=2) as ropep,
            tc.tile_pool(name="ep", bufs=4) as ep,
            tc.tile_pool(name="etmpp", bufs=2) as etmpp,
            tc.tile_pool(name="attp", bufs=2) as attp,
            tc.tile_pool(name="recipp", bufs=2) as recipp,
            tc.tile_pool(name="rbcp", bufs=2) as rbcp,
            tc.tile_pool(name="gp", bufs=2) as gp,
            tc.tile_pool(name="oobp", bufs=2) as oobp,
            tc.tile_pool(name="psacc", bufs=3, space="PSUM") as psacc,
            tc.tile_pool(name="pss", bufs=2, space="PSUM") as pss,
            tc.tile_pool(name="psu", bufs=2, space="PSUM") as psu,
            tc.tile_pool(name="pssum", bufs=1, space="PSUM") as pssum,
        ):
            # ---- constants ----
            wq_sb = constp.tile([128, KT, 256], BF16)
            wk_sb = constp.tile([128, KT, 128], BF16)
            wv_sb = constp.tile([128, KT, 128], BF16)
            wo_sb = constp.tile([128, KT, 256], BF16)
            cos_sb = constp.tile([128, N], BF16)
            sin_sb = constp.tile([128, N], BF16)
            mask_sb = constp.tile([128, 2048], BF16)
            ones_sb = constp.tile([128, 1], BF16)
            id_sb = constp.tile([128, 128], BF16)
            nc.scalar.dma_start(wq_sb[:, 0:4, :], wq[:, 0:4, :])
            nc.vector.memset(ones_sb[:], 1.0)

            def early_consts():
                for c in range(1, 4):
                    nc.scalar.dma_start(wq_sb[:, c * 4:(c + 1) * 4, :],
                                        wq[:, c * 4:(c + 1) * 4, :])
                nc.scalar.dma_start(wk_sb[:], wk[:])
                nc.scalar.dma_start(wv_sb[:], wv[:])
                nc.scalar.dma_start(id_sb[:], ident[:])
                nc.scalar.dma_start(cos_sb[:], cost[:])
                nc.scalar.dma_start(sin_sb[:], sint[:])
                nc.scalar.dma_start(mask_sb[:], mask[:])

            def late_consts():
                nc.scalar.dma_start(wo_sb[:], wo[:])

            q_sb = [persist.tile([128, NT], BF16, name=f"q{h}_sb") for h in range(2)]
            k_sb = persist.tile([128, NT], BF16)
            v_sb = persist.tile([128, NT], BF16)

            xt_r = xt.rearrange("(t p) n -> p t n", p=128)

            xblks = {}

            def xblk_load(nb):
                col0 = nb * 512
                xblk = xtp.tile([128, KT, 512], BF16, tag="xblk", name=f"xblk_{nb}")
                ring = nc.sync if nb % 2 == 0 else nc.scalar
                csz = 4 if nb == 0 else 8
                for c0 in range(0, KT, csz):
                    ring.dma_start(xblk[:, c0:c0 + csz, :],
                                   xt_r[:, c0:c0 + csz, col0:col0 + 512])
                xblks[nb] = xblk

            def rope_chunk(raw, dst, c0, col0):
                rot = ropep.tile([128, 512], BF16, tag="rot")
                nc.sync.dma_start(rot[0:64, :], raw[64:128, :])
                nc.sync.dma_start(rot[64:128, :], raw[0:64, :])
                t1 = ropep.tile([128, 512], BF16, tag="t1")
                nc.vector.tensor_mul(t1[:], raw[:], cos_sb[:, c0:c0 + 512])
                t2 = ropep.tile([128, 512], BF16, tag="t2")
                nc.vector.tensor_mul(t2[:], rot[:], sin_sb[:, c0:c0 + 512])
                nc.vector.tensor_add(dst[:, col0:col0 + 512], t1[:], t2[:])

            def qkv_block(nb):
                col0 = nb * 512
                c0 = (nb % 2) * 512
                xblk = xblks[nb]
                if nb == 1:
                    late_consts()

                def accum(dst_ps, w_sb, msl):
                    for k0 in range(0, KT, 4):
                        for kt in range(k0, k0 + 4):
                            nc.tensor.matmul(
                                dst_ps, w_sb[:, kt, msl], xblk[:, kt, :],
                                start=(kt == 0), stop=(kt == KT - 1))
                        yield

                for m in range(2):
                    raw = qkrawp.tile([128, 512], BF16, tag=f"qraw{m}",
                                      name=f"qraw{m}_{nb}")
                    q_ps = psacc.tile([128, 512], F32, tag="psacc",
                                      name=f"q_ps_{nb}_{m}")
                    yield from accum(q_ps[:], wq_sb,
                                     slice(m * 128, (m + 1) * 128))
                    nc.scalar.activation(raw[:], q_ps[:],
                                         mybir.ActivationFunctionType.Copy)
                    yield
                    rope_chunk(raw, q_sb[m], c0, col0)
                kraw = qkrawp.tile([128, 512], BF16, tag="kraw", name=f"kraw_{nb}")
                k_ps = psacc.tile([128, 512], F32, tag="psacc", name=f"k_ps_{nb}")
                yield from accum(k_ps[:], wk_sb, slice(0, 128))
                nc.scalar.activation(kraw[:], k_ps[:],
                                     mybir.ActivationFunctionType.Copy)
                yield
                v_ps = psacc.tile([128, 512], F32, tag="psacc", name=f"v_ps_{nb}")
                yield from accum(v_ps[:], wv_sb, slice(0, 128))
                vraw = ropep.tile([128, 512], BF16, tag="vraw")
                nc.scalar.activation(vraw[:], v_ps[:],
                                     mybir.ActivationFunctionType.Copy)
                yield
                rope_chunk(kraw, k_sb, c0, col0)
                vt_ps = psacc.tile([128, 1024], BF16, tag="psacc",
                                   name=f"vt_ps_{nb}")
                for i in range(4):
                    nc.tensor.matmul(vt_ps[:, i * 128:(i + 1) * 128],
                                     vraw[:, i * 128:(i + 1) * 128], id_sb[:],
                                     is_transpose=True, skip_group_check=True)
                nc.scalar.activation(v_sb[:, col0:col0 + 512], vt_ps[:, 0:512],
                                     mybir.ActivationFunctionType.Copy)
                yield

            def att_ib(b, ib):
                icol = b * N + ib * 512
                cnt = 4 * ib + 4
                for h in range(2):
                    qh = q_sb[h]
                    att = attp.tile([128, 512], BF16, tag="att",
                                    name=f"att_{b}_{ib}_{h}")
                    u_ps = psu.tile([128, 512], F32, tag="psu",
                                    name=f"u_ps_{b}_{ib}_{h}")
                    sum_ps = pssum.tile([1, 512], F32, tag="pssum",
                                        name=f"sum_ps_{b}_{ib}_{h}")

                    def c_lo(jt):
                        r = jt - 4 * ib
                        return 128 * r if r > 0 else 0

                    def s_mm(jt):
                        s_ps = pss.tile([128, 512], F32, tag="pss",
                                        name=f"s_ps_{b}_{ib}_{h}_{jt}")
                        jcol = b * N + jt * 128
                        c0 = c_lo(jt)
                        nc.tensor.matmul(
                            s_ps[:, c0:512], k_sb[:, jcol:jcol + 128],
                            qh[:, icol + c0:icol + 512],
                            start=True, stop=True)
                        return s_ps

                    def e_of(jt, s_ps):
                        r = jt - 4 * ib
                        c0 = c_lo(jt)
                        e = ep.tile([128, 512], BF16, tag="e",
                                    name=f"e_{b}_{ib}_{h}_{jt}")
                        if r >= 0:
                            etmp = etmpp.tile([128, 512], BF16, tag="etmp")
                            nc.scalar.activation(
                                etmp[:, c0:512], s_ps[:, c0:512],
                                mybir.ActivationFunctionType.Exp, scale=SCALE)
                            nc.vector.tensor_mul(
                                e[:, c0:512], etmp[:, c0:512],
                                mask_sb[:, r * 512 + c0:(r + 1) * 512])
                        else:
                            nc.scalar.activation(
                                e[:], s_ps[:],
                                mybir.ActivationFunctionType.Exp, scale=SCALE)
                        return e

                    s_tiles = {0: s_mm(0), 1: s_mm(1)}
                    for jt in range(cnt):
                        e = e_of(jt, s_tiles.pop(jt))
                        if jt + 2 < cnt:
                            s_tiles[jt + 2] = s_mm(jt + 2)
                        tt = b * 8 + jt
                        c0 = c_lo(jt)
                        nc.tensor.matmul(
                            u_ps[:, c0:512],
                            v_sb[:, tt * 128:(tt + 1) * 128], e[:, c0:512],
                            start=(jt == 0), stop=(jt == cnt - 1),
                            skip_group_check=True)
                        nc.tensor.matmul(
                            sum_ps[:, c0:512], ones_sb[:], e[:, c0:512],
                            start=(jt == 0), stop=(jt == cnt - 1),
                            skip_group_check=True)
                        yield
                    recip = recipp.tile([1, 512], F32, tag="recip")
                    nc.vector.reciprocal_approx_fast(out=recip[:], in_=sum_ps[:])
                    rbc = rbcp.tile([128, 512], F32, tag="rbc")
                    nc.gpsimd.partition_broadcast(rbc[:], recip[:])
                    nc.vector.tensor_mul(att[:], u_ps[:], rbc[:])
                    nc.gpsimd.dma_start(
                        ag_in[(b, ib)][h * 128:(h + 1) * 128, :], att[:])
                    yield

            def allgather(b, ib):
                nc.gpsimd.collective_compute(
                    "AllGather",
                    mybir.AluOpType.bypass,
                    replica_groups=[list(range(N_CORES))],
                    ins=[ag_in[(b, ib)][:].opt()],
                    outs=[ag_out[(b, ib)][:].opt()],
                )

            g_tiles = {}

            def g_prefetch(b, ib):
                g_tiles[(b, ib)] = gp.tile([128, KT, 512], BF16, tag="g",
                                           name=f"g_{b}_{ib}")
                for c0 in range(0, KT, 8):
                    nc.gpsimd.dma_start(g_tiles[(b, ib)][:, c0:c0 + 8, :],
                                        ag_out_r[(b, ib)][:, c0:c0 + 8, :])

            def oproj_ib(b, ib):
                g = g_tiles.pop((b, ib))
                for m in range(2):
                    o_ps = psacc.tile([128, 512], F32, tag="psacc",
                                      name=f"o_ps_{b}_{ib}_{m}")
                    for k0 in range(0, KT, 4):
                        for kt in range(k0, k0 + 4):
                            nc.tensor.matmul(
                                o_ps[:], wo_sb[:, kt, m * 128:(m + 1) * 128],
                                g[:, kt, :], start=(kt == 0),
                                stop=(kt == KT - 1))
                        yield
                    osb = oobp.tile([128, 512], BF16, tag="osb",
                                    name=f"osb_{b}_{ib}_{m}")
                    nc.vector.tensor_copy(osb[:], o_ps[:])
                    nc.gpsimd.dma_start(
                        out[m * 128:(m + 1) * 128,
                            b * N + ib * 512:b * N + (ib + 1) * 512], osb[:])
                    yield

            def drain(gen):
                for _ in gen:
                    pass

            def chain(*gens):
                for g in gens:
                    yield from g

            def interleave(gen_a, gen_b, na=1, nb=1, prime_b=0):
                for _ in range(prime_b):
                    try:
                        next(gen_b)
                    except StopIteration:
                        break
                alive = [gen_a, gen_b]
                while alive:
                    for g in list(alive):
                        steps = na if g is gen_a else nb
                        for _ in range(steps):
                            try:
                                next(g)
                            except StopIteration:
                                if g in alive:
                                    alive.remove(g)
                                break

            # ---- schedule ----
            xblk_load(0)
            early_consts()
            xblk_load(1)
            drain(qkv_block(0))
            xblk_load(2)
            interleave(att_ib(0, 0), qkv_block(1), na=1, nb=2, prime_b=4)
            allgather(0, 0)
            xblk_load(3)
            interleave(att_ib(0, 1), qkv_block(2), na=1, nb=1, prime_b=4)
            allgather(0, 1)
            xblk_load(4)
            interleave(att_ib(1, 0), qkv_block(3), na=1, nb=2, prime_b=4)
            allgather(1, 0)
            g_prefetch(0, 0)
            xblk_load(5)
            interleave(att_ib(1, 1), qkv_block(4), na=1, nb=1, prime_b=4)
            allgather(1, 1)
            g_prefetch(0, 1)
            xblk_load(6)
            interleave(att_ib(2, 0), chain(qkv_block(5), oproj_ib(0, 0)),
                       na=1, nb=2, prime_b=4)
            allgather(2, 0)
            g_prefetch(1, 0)
            xblk_load(7)
            interleave(att_ib(2, 1), chain(qkv_block(6), oproj_ib(0, 1)),
                       na=1, nb=1, prime_b=4)
            allgather(2, 1)
            g_prefetch(1, 1)
            interleave(att_ib(3, 0), chain(qkv_block(7), oproj_ib(1, 0)),
                       na=1, nb=2, prime_b=4)
            allgather(3, 0)
            g_prefetch(2, 0)
            g_prefetch(2, 1)
            g_prefetch(3, 0)
            interleave(att_ib(3, 1),
                       chain(oproj_ib(1, 1), oproj_ib(2, 0), oproj_ib(2, 1),
                             oproj_ib(3, 0)),
                       na=1, nb=1, prime_b=2)
            allgather(3, 1)
            g_prefetch(3, 1)
            drain(oproj_ib(3, 1))

    nc.compile()
    _NC_CACHE["nc"] = nc
    return nc


def _host_prep(x, Wq, Wk, Wv, Wo, head_scale):
    bf = ml_dtypes.bfloat16
    xt = np.ascontiguousarray(x.reshape(NT, D).T).astype(bf)

    hs = np.asarray(head_scale).reshape(16)
    wo_s = (np.asarray(Wo) * np.repeat(hs, DH)[:, None]).astype(np.float32)

    def ktile(w):
        m = w.shape[1]
        return np.ascontiguousarray(
            w.reshape(KT, 128, m).transpose(1, 0, 2)).astype(bf)

    inv_freq = (1.0 / (10000.0 ** (np.arange(0, DH, 2, dtype=np.float64) / DH)))
    freqs = np.arange(N, dtype=np.float64)[:, None] * inv_freq[None, :]
    emb = np.concatenate([freqs, freqs], axis=-1)
    cosT = np.ascontiguousarray(np.cos(emb).T).astype(bf)
    sinT = np.sin(emb).T
    sign = np.where(np.arange(DH) < 64, -1.0, 1.0)[:, None]
    sinT = np.ascontiguousarray(sinT * sign).astype(bf)

    p = np.arange(128)[:, None]
    c = np.arange(512)[None, :]
    masks = [(c >= p + 128 * r).astype(np.float32) for r in range(4)]
    mask = np.concatenate(masks, axis=1).astype(bf)

    idm = np.eye(128, dtype=np.float32).astype(bf)

    in_maps = []
    for core in range(N_CORES):
        kv = core // 2
        in_maps.append({
            "xt": xt,
            "wq": ktile(np.asarray(Wq)[:, core * 256:(core + 1) * 256]),
            "wk": ktile(np.asarray(Wk)[:, kv * 128:(kv + 1) * 128]),
            "wv": ktile(np.asarray(Wv)[:, kv * 128:(kv + 1) * 128]),
            "wo": ktile(wo_s[:, core * 256:(core + 1) * 256]),
            "cost": cosT,
            "sint": sinT,
            "mask": mask,
            "ident": idm,
        })
    return in_maps


def kernel(x, Wq, Wk, Wv, Wo, head_scale, _run_kwargs=None):
    nc = build_nc()
    in_maps = _host_prep(x, Wq, Wk, Wv, Wo, head_scale)
    res = run_bass_kernel_spmd(
        nc, in_maps, core_ids=list(range(N_CORES)), **(_run_kwargs or {})
    )
    outT = np.concatenate(
        [res.results[c]["out"].astype(np.float32) for c in range(N_CORES)], axis=0)
    full = np.ascontiguousarray(outT.T).reshape(B, N, D)
    if _run_kwargs:
        kernel.last_results = res
    return full


# revision 24
# speedup vs baseline: 1.0282x; 1.0282x over previous
"""Trainium2 8-core kernel for causal GQA attention (nn_Attention_90967407329949).

v1 architecture (no kv dedup) + stall fixes:
 - ag_in writes on gpsimd ring (off the big-load lanes)
 - oproj schedule shifted one segment later (dodges the slow first AllGather)
 - tail reordered: oproj(3,0) interleaved with att(3,1); only oproj(3,1)
   after the last collective
 - startup: wq chunk0 + xblk0 first on their lanes

Distribution: tensor-parallel over query heads (2 q-heads + shared kv-head
per core); per (batch, half) AllGathers of attention outputs; each core
computes a 256-column slice of the output projection.

V tiles are transposed on the PE (identity matmul), NOT dma_start_transpose
(Tile serializes DMA transposes with collectives).
"""

import numpy as np
import ml_dtypes

import concourse.bacc as bacc
import concourse.mybir as mybir
import concourse.tile as tile
from concourse.bass_utils import run_bass_kernel_spmd

BF16 = mybir.dt.bfloat16
F32 = mybir.dt.float32

N_CORES = 8
B = 4
N = 1024
NT = B * N
D = 2048
DH = 128
KT = D // 128
SCALE = 1.0 / np.sqrt(DH)

_NC_CACHE = {}


def build_nc():
    if "nc" in _NC_CACHE:
        return _NC_CACHE["nc"]
    nc = bacc.Bacc("TRN2", target_bir_lowering=False, debug=False, num_devices=N_CORES)

    xt = nc.dram_tensor("xt", [D, NT], BF16, kind="ExternalInput")
    wq = nc.dram_tensor("wq", [128, KT, 256], BF16, kind="ExternalInput")
    wk = nc.dram_tensor("wk", [128, KT, 128], BF16, kind="ExternalInput")
    wv = nc.dram_tensor("wv", [128, KT, 128], BF16, kind="ExternalInput")
    wo = nc.dram_tensor("wo", [128, KT, 256], BF16, kind="ExternalInput")
    cost = nc.dram_tensor("cost", [128, N], BF16, kind="ExternalInput")
    sint = nc.dram_tensor("sint", [128, N], BF16, kind="ExternalInput")
    mask = nc.dram_tensor("mask", [128, 2048], BF16, kind="ExternalInput")
    ident = nc.dram_tensor("ident", [128, 128], BF16, kind="ExternalInput")
    out = nc.dram_tensor("out", [256, NT], BF16, kind="ExternalOutput")

    SEGS = [(b, ib) for b in range(B) for ib in range(2)]
    ag_in = {s: nc.dram_tensor(f"agi{s[0]}{s[1]}", [256, 512], BF16) for s in SEGS}
    ag_out = {s: nc.dram_tensor(f"ago{s[0]}{s[1]}", [D, 512], BF16,
                                addr_space="Shared") for s in SEGS}
    ag_out_r = {s: t.rearrange("(t p) n -> p t n", p=128) for s, t in ag_out.items()}

    with tile.TileContext(nc) as tc:
        with (
            tc.tile_pool(name="const", bufs=1) as constp,
            tc.tile_pool(name="persist", bufs=1) as persist,
            tc.tile_pool(name="xtp", bufs=3) as xtp,
            tc.tile_pool(name="qkraw", bufs=2) as qkrawp,
            tc.tile_pool(name="rope", bufs=2) as ropep,
            tc.tile_pool(name="ep", bufs=4) as ep,
            tc.tile_pool(name="etmpp", bufs=2) as etmpp,
            tc.tile_pool(name="attp", bufs=2) as attp,
            tc.tile_pool(name="recipp", bufs=2) as recipp,
            tc.tile_pool(name="rbcp", bufs=2) as rbcp,
            tc.tile_pool(name="gp", bufs=2) as gp,
            tc.tile_pool(name="oobp", bufs=2) as oobp,
            tc.tile_pool(name="psacc", bufs=3, space="PSUM") as psacc,
            tc.tile_pool(name="pss", bufs=2, space="PSUM") as pss,
            tc.tile_pool(name="psu", bufs=2, space="PSUM") as psu,
            tc.tile_pool(name="pssum", bufs=1, space="PSUM") as pssum,
        ):
            # ---- constants ----
            wq_sb = constp.tile([128, KT, 256], BF16)
            wk_sb = constp.tile([128, KT, 128], BF16)
            wv_sb = constp.tile([128, KT, 128], BF16)
            wo_sb = constp.tile([128, KT, 256], BF16)
            cos_sb = constp.tile([128, N], BF16)
            sin_sb = constp.tile([128, N], BF16)
            mask_sb = constp.tile([128, 2048], BF16)
            ones_sb = constp.tile([128, 1], BF16)
            id_sb = constp.tile([128, 128], BF16)
            nc.scalar.dma_start(wq_sb[:, 0:4, :], wq[:, 0:4, :])
            nc.vector.memset(ones_sb[:], 1.0)

            def early_consts():
                for c in range(1, 4):
                    nc.scalar.dma_start(wq_sb[:, c * 4:(c + 1) * 4, :],
                                        wq[:, c * 4:(c + 1) * 4, :])
                nc.scalar.dma_start(wk_sb[:], wk[:])
                nc.scalar.dma_start(wv_sb[:], wv[:])
                nc.scalar.dma_start(id_sb[:], ident[:])
                nc.scalar.dma_start(cos_sb[:], cost[:])
                nc.scalar.dma_start(sin_sb[:], sint[:])
                nc.scalar.dma_start(mask_sb[:], mask[:])

            def late_consts():
                nc.scalar.dma_start(wo_sb[:], wo[:])

            q_sb = [persist.tile([128, NT], BF16, name=f"q{h}_sb") for h in range(2)]
            k_sb = persist.tile([128, NT], BF16)
            v_sb = persist.tile([128, NT], BF16)

            xt_r = xt.rearrange("(t p) n -> p t n", p=128)

            xblks = {}

            def xblk_load(nb):
                col0 = nb * 512
                xblk = xtp.tile([128, KT, 512], BF16, tag="xblk", name=f"xblk_{nb}")
                ring = nc.sync if nb % 2 == 0 else nc.scalar
                csz = 4 if nb == 0 else 8
                for c0 in range(0, KT, csz):
                    ring.dma_start(xblk[:, c0:c0 + csz, :],
                                   xt_r[:, c0:c0 + csz, col0:col0 + 512])
                xblks[nb] = xblk

            def rope_chunk(raw, dst, c0, col0):
                rot = ropep.tile([128, 512], BF16, tag="rot")
                nc.sync.dma_start(rot[0:64, :], raw[64:128, :])
                nc.sync.dma_start(rot[64:128, :], raw[0:64, :])
                t1 = ropep.tile([128, 512], BF16, tag="t1")
                nc.vector.tensor_mul(t1[:], raw[:], cos_sb[:, c0:c0 + 512])
                t2 = ropep.tile([128, 512], BF16, tag="t2")
                nc.vector.tensor_mul(t2[:], rot[:], sin_sb[:, c0:c0 + 512])
                nc.vector.tensor_add(dst[:, col0:col0 + 512], t1[:], t2[:])

            def qkv_block(nb):
                col0 = nb * 512
                c0 = (nb % 2) * 512
                xblk = xblks[nb]
                if nb == 1:
                    late_consts()

                def accum(dst_ps, w_sb, msl):
                    for k0 in range(0, KT, 4):
                        for kt in range(k0, k0 + 4):
                            nc.tensor.matmul(
                                dst_ps, w_sb[:, kt, msl], xblk[:, kt, :],
                                start=(kt == 0), stop=(kt == KT - 1))
                        yield

                for m in range(2):
                    raw = qkrawp.tile([128, 512], BF16, tag=f"qraw{m}",
                                      name=f"qraw{m}_{nb}")
                    q_ps = psacc.tile([128, 512], F32, tag="psacc",
                                      name=f"q_ps_{nb}_{m}")
                    yield from accum(q_ps[:], wq_sb,
                                     slice(m * 128, (m + 1) * 128))
                    nc.scalar.activation(raw[:], q_ps[:],
                                         mybir.ActivationFunctionType.Copy)
                    yield
                    rope_chunk(raw, q_sb[m], c0, col0)
                kraw = qkrawp.tile([128, 512], BF16, tag="kraw", name=f"kraw_{nb}")
                k_ps = psacc.tile([128, 512], F32, tag="psacc", name=f"k_ps_{nb}")
                yield from accum(k_ps[:], wk_sb, slice(0, 128))
                nc.scalar.activation(kraw[:], k_ps[:],
                                     mybir.ActivationFunctionType.Copy)
                yield
                v_ps = psacc.tile([128, 512], F32, tag="psacc", name=f"v_ps_{nb}")
                yield from accum(v_ps[:], wv_sb, slice(0, 128))
                vraw = ropep.tile([128, 512], BF16, tag="vraw")
                nc.scalar.activation(vraw[:], v_ps[:],
                                     mybir.ActivationFunctionType.Copy)
                yield
                rope_chunk(kraw, k_sb, c0, col0)
                vt_ps = psacc.tile([128, 1024], BF16, tag="psacc",
                                   name=f"vt_ps_{nb}")
                for i in range(4):
                    nc.tensor.matmul(vt_ps[:, i * 128:(i + 1) * 128],
                                     vraw[:, i * 128:(i + 1) * 128], id_sb[:],
                                     is_transpose=True, skip_group_check=True)
                nc.scalar.activation(v_sb[:, col0:col0 + 512], vt_ps[:, 0:512],
                                     mybir.ActivationFunctionType.Copy)
                yield

            def att_ib(b, ib):
                icol = b * N + ib * 512
                cnt = 4 * ib + 4
                for h in range(2):
                    qh = q_sb[h]
                    att = attp.tile([128, 512], BF16, tag="att",
                                    name=f"att_{b}_{ib}_{h}")
                    u_ps = psu.tile([128, 512], F32, tag="psu",
                                    name=f"u_ps_{b}_{ib}_{h}")
                    sum_ps = pssum.tile([1, 512], F32, tag="pssum",
                                        name=f"sum_ps_{b}_{ib}_{h}")

                    def c_lo(jt):
                        r = jt - 4 * ib
                        return 128 * r if r > 0 else 0

                    def s_mm(jt):
                        s_ps = pss.tile([128, 512], F32, tag="pss",
                                        name=f"s_ps_{b}_{ib}_{h}_{jt}")
                        jcol = b * N + jt * 128
                        c0 = c_lo(jt)
                        nc.tensor.matmul(
                            s_ps[:, c0:512], k_sb[:, jcol:jcol + 128],
                            qh[:, icol + c0:icol + 512],
                            start=True, stop=True)
                        return s_ps

                    def e_of(jt, s_ps):
                        r = jt - 4 * ib
                        c0 = c_lo(jt)
                        e = ep.tile([128, 512], BF16, tag="e",
                                    name=f"e_{b}_{ib}_{h}_{jt}")
                        if r >= 0:
                            etmp = etmpp.tile([128, 512], BF16, tag="etmp")
                            nc.scalar.activation(
                                etmp[:, c0:512], s_ps[:, c0:512],
                                mybir.ActivationFunctionType.Exp, scale=SCALE)
                            nc.vector.tensor_mul(
                                e[:, c0:512], etmp[:, c0:512],
                                mask_sb[:, r * 512 + c0:(r + 1) * 512])
                        else:
                            nc.scalar.activation(
                                e[:], s_ps[:],
                                mybir.ActivationFunctionType.Exp, scale=SCALE)
                        return e

                    s_tiles = {0: s_mm(0), 1: s_mm(1)}
                    for jt in range(cnt):
                        e = e_of(jt, s_tiles.pop(jt))
                        if jt + 2 < cnt:
                            s_tiles[jt + 2] = s_mm(jt + 2)
                        tt = b * 8 + jt
                        c0 = c_lo(jt)
                        nc.tensor.matmul(
                            u_ps[:, c0:512],
                            v_sb[:, tt * 128:(tt + 1) * 128], e[:, c0:512],
                            start=(jt == 0), stop=(jt == cnt - 1),
                            skip_group_check=True)
                        nc.tensor.matmul(
                            sum_ps[:, c0:512], ones_sb[:], e[:, c0:512],
                            start=(jt == 0), stop=(jt == cnt - 1),
                            skip_group_check=True)
                        yield
                    recip = recipp.tile([1, 512], F32, tag="recip")
                    nc.vector.reciprocal_approx_fast(out=recip[:], in_=sum_ps[:])
                    rbc = rbcp.tile([128, 512], F32, tag="rbc")
                    nc.gpsimd.partition_broadcast(rbc[:], recip[:])
                    nc.vector.tensor_mul(att[:], u_ps[:], rbc[:])
                    nc.gpsimd.dma_start(
                        ag_in[(b, ib)][h * 128:(h + 1) * 128, :], att[:])
                    yield

            def allgather(b, ib):
                nc.gpsimd.collective_compute(
                    "AllGather",
                    mybir.AluOpType.bypass,
                    replica_groups=[list(range(N_CORES))],
                    ins=[ag_in[(b, ib)][:].opt()],
                    outs=[ag_out[(b, ib)][:].opt()],
                )

            g_tiles = {}

            def g_prefetch(b, ib):
                g_tiles[(b, ib)] = gp.tile([128, KT, 512], BF16, tag="g",
                                           name=f"g_{b}_{ib}")
                for c0 in range(0, KT, 8):
                    nc.gpsimd.dma_start(g_tiles[(b, ib)][:, c0:c0 + 8, :],
                                        ag_out_r[(b, ib)][:, c0:c0 + 8, :])

            def oproj_ib(b, ib):
                g = g_tiles.pop((b, ib))
                for m in range(2):
                    o_ps = psacc.tile([128, 512], F32, tag="psacc",
                                      name=f"o_ps_{b}_{ib}_{m}")
                    for k0 in range(0, KT, 4):
                        for kt in range(k0, k0 + 4):
                            nc.tensor.matmul(
                                o_ps[:], wo_sb[:, kt, m * 128:(m + 1) * 128],
                                g[:, kt, :], start=(kt == 0),
                                stop=(kt == KT - 1))
                        yield
                    osb = oobp.tile([128, 512], BF16, tag="osb",
                                    name=f"osb_{b}_{ib}_{m}")
                    nc.vector.tensor_copy(osb[:], o_ps[:])
                    nc.gpsimd.dma_start(
                        out[m * 128:(m + 1) * 128,
                            b * N + ib * 512:b * N + (ib + 1) * 512], osb[:])
                    yield

            def drain(gen):
                for _ in gen:
                    pass

            def chain(*gens):
                for g in gens:
                    yield from g

            def interleave(gen_a, gen_b, na=1, nb=1, prime_b=0):
                for _ in range(prime_b):
                    try:
                        next(gen_b)
                    except StopIteration:
                        break
                alive = [gen_a, gen_b]
                while alive:
                    for g in list(alive):
                        steps = na if g is gen_a else nb
                        for _ in range(steps):
                            try:
                                next(g)
                            except StopIteration:
                                if g in alive:
                                    alive.remove(g)
                                break

            # ---- schedule ----
            xblk_load(0)
            early_consts()
            xblk_load(1)
            drain(qkv_block(0))
            xblk_load(2)
            interleave(att_ib(0, 0), qkv_block(1), na=1, nb=3, prime_b=4)
            allgather(0, 0)
            xblk_load(3)
            interleave(att_ib(0, 1), qkv_block(2), na=1, nb=1, prime_b=4)
            allgather(0, 1)
            xblk_load(4)
            interleave(att_ib(1, 0), qkv_block(3), na=1, nb=3, prime_b=4)
            allgather(1, 0)
            g_prefetch(0, 0)
            xblk_load(5)
            interleave(att_ib(1, 1), qkv_block(4), na=1, nb=1, prime_b=4)
            allgather(1, 1)
            g_prefetch(0, 1)
            xblk_load(6)
            interleave(att_ib(2, 0), chain(qkv_block(5), oproj_ib(0, 0)),
                       na=1, nb=4, prime_b=4)
            allgather(2, 0)
            g_prefetch(1, 0)
            xblk_load(7)
            interleave(att_ib(2, 1), chain(qkv_block(6), oproj_ib(0, 1)),
                       na=1, nb=2, prime_b=4)
            allgather(2, 1)
            g_prefetch(1, 1)
            interleave(att_ib(3, 0), chain(qkv_block(7), oproj_ib(1, 0)),
                       na=1, nb=4, prime_b=4)
            allgather(3, 0)
            g_prefetch(2, 0)
            g_prefetch(2, 1)
            g_prefetch(3, 0)
            interleave(att_ib(3, 1),
                       chain(oproj_ib(1, 1), oproj_ib(2, 0), oproj_ib(2, 1),
                             oproj_ib(3, 0)),
                       na=1, nb=3, prime_b=2)
            allgather(3, 1)
            g_prefetch(3, 1)
            drain(oproj_ib(3, 1))

    nc.compile()
    _NC_CACHE["nc"] = nc
    return nc


def _host_prep(x, Wq, Wk, Wv, Wo, head_scale):
    bf = ml_dtypes.bfloat16
    xt = np.ascontiguousarray(x.reshape(NT, D).T).astype(bf)

    hs = np.asarray(head_scale).reshape(16)
    wo_s = (np.asarray(Wo) * np.repeat(hs, DH)[:, None]).astype(np.float32)

    def ktile(w):
        m = w.shape[1]
        return np.ascontiguousarray(
            w.reshape(KT, 128, m).transpose(1, 0, 2)).astype(bf)

    inv_freq = (1.0 / (10000.0 ** (np.arange(0, DH, 2, dtype=np.float64) / DH)))
    freqs = np.arange(N, dtype=np.float64)[:, None] * inv_freq[None, :]
    emb = np.concatenate([freqs, freqs], axis=-1)
    cosT = np.ascontiguousarray(np.cos(emb).T).astype(bf)
    sinT = np.sin(emb).T
    sign = np.where(np.arange(DH) < 64, -1.0, 1.0)[:, None]
    sinT = np.ascontiguousarray(sinT * sign).astype(bf)

    p = np.arange(128)[:, None]
    c = np.arange(512)[None, :]
    masks = [(c >= p + 128 * r).astype(np.float32) for r in range(4)]
    mask = np.concatenate(masks, axis=1).astype(bf)

    idm = np.eye(128, dtype=np.float32).astype(bf)

    in_maps = []
    for core in range(N_CORES):
        kv = core // 2
        in_maps.append({
            "xt": xt,
            "wq": ktile(np.asarray(Wq)[:, core * 256:(core + 1) * 256]),
            "wk": ktile(np.asarray(Wk)[:, kv * 128:(kv + 1) * 128]),
            "wv": ktile(np.asarray(Wv)[:, kv * 128:(kv + 1) * 128]),
            "wo": ktile(wo_s[:, core * 256:(core + 1) * 256]),
            "cost": cosT,
            "sint": sinT,
            "mask": mask,
            "ident": idm,
        })
    return in_maps


def kernel(x, Wq, Wk, Wv, Wo, head_scale, _run_kwargs=None):
    nc = build_nc()
    in_maps = _host_prep(x, Wq, Wk, Wv, Wo, head_scale)
    res = run_bass_kernel_spmd(
        nc, in_maps, core_ids=list(range(N_CORES)), **(_run_kwargs or {})
    )
    outT = np.concatenate(
        [res.results[c]["out"].astype(np.float32) for c in range(N_CORES)], axis=0)
    full = np.ascontiguousarray(outT.T).reshape(B, N, D)
    if _run_kwargs:
        kernel.last_results = res
    return full
